# revision 1
# baseline (speedup 1.0000x reference)
"""MoE layer (8 experts, top-2, capacity 2560) on 8 Trainium2 NeuronCores.

Expert-parallel: one expert per core. Host does gating/routing (the
data-dependent "sharding"/dispatch step) and the final weighted combine;
each core runs the dense expert FFN  relu(buf @ w1 + b1) @ w2 + b2  for
its expert over the capacity-padded dispatch buffer.

Device kernel layout (per core, all shapes hardcoded):
  buf^T is processed in 5 chunks of 512 tokens. For each chunk:
    layer 1: for each of 64 hidden tiles (128 rows of H), accumulate 16
      matmuls (contraction D=2048 in 128-tiles) into one PSUM bank, then
      ACT relu+bias into an SBUF-resident hidden tile [128, 512] (bf16).
    layer 2: 4 sweeps of 4 output d-tiles; each sweep accumulates 64
      matmuls per d-tile (contraction H=8192) across 4 PSUM banks, then
      ACT copy+bias to SBUF and DMA out.
  Weights stream from HBM (re-read once per chunk); activations stay in
  SBUF. bf16 multiplies, fp32 PSUM accumulation.
"""

import numpy as np
import ml_dtypes

import concourse.bacc as bacc
import concourse.mybir as mybir
import concourse.tile as tile
from concourse import bass_utils

F32 = mybir.dt.float32
BF16 = mybir.dt.bfloat16
AF = mybir.ActivationFunctionType

# Problem constants (from the reference module).
NUM_EXPERTS = 8
TOP_K = 2
D = 2048          # d_model
H = 8192          # d_hidden
B, S = 4, 2048
T = B * S         # 8192 tokens
CAP = 2560        # ceil(T*K/E * 1.25)

CC = 5            # token chunks per expert
CHUNK = 512       # tokens per chunk (CC*CHUNK == CAP)
DT = 16           # d tiles (DT*128 == D)
HT = 64           # h tiles (HT*128 == H)
DQ = 4            # layer-2 sweeps (DQ * 4 d-tiles == DT)

_CACHE = {}


def _build_nc():
    nc = bacc.Bacc("TRN2", target_bir_lowering=False, debug=False)
    bufx = nc.dram_tensor("bufx", [CC, 128, DT, CHUNK], BF16, kind="ExternalInput")
    w1x = nc.dram_tensor("w1x", [HT, 128, DT, 128], BF16, kind="ExternalInput")
    w2x = nc.dram_tensor("w2x", [HT, 128, DQ, 4, 128], BF16, kind="ExternalInput")
    b1x = nc.dram_tensor("b1x", [128, HT], F32, kind="ExternalInput")
    b2x = nc.dram_tensor("b2x", [128, DT], F32, kind="ExternalInput")
    outx = nc.dram_tensor("outx", [CC, DT, 128, CHUNK], F32, kind="ExternalOutput")

    with tile.TileContext(nc) as tc:
        with (
            tc.tile_pool(name="consts", bufs=1) as consts,
            tc.tile_pool(name="bufp", bufs=2) as bufp,
            tc.tile_pool(name="w1p", bufs=3) as w1p,
            tc.tile_pool(name="w2p", bufs=6) as w2p,
            tc.tile_pool(name="hp", bufs=1) as hp,
            tc.tile_pool(name="outp", bufs=4) as outp,
            tc.tile_pool(name="ps1", bufs=4, space="PSUM") as ps1,
            tc.tile_pool(name="ps2", bufs=4, space="PSUM") as ps2,
        ):
            b1_sb = consts.tile([128, HT], F32)
            b2_sb = consts.tile([128, DT], F32)
            nc.sync.dma_start(b1_sb[:], b1x[:])
            nc.sync.dma_start(b2_sb[:], b2x[:])

            for cc in range(CC):
                buf_sb = bufp.tile([128, DT, CHUNK], BF16)
                nc.sync.dma_start(buf_sb[:], bufx[cc])
                hT = hp.tile([128, HT, CHUNK], BF16)

                # ---- layer 1: hT[ht] = relu(w1[:,ht]^T @ bufT + b1[ht]) ----
                for ht in range(HT):
                    w1_sb = w1p.tile([128, DT, 128], BF16)
                    nc.sync.dma_start(w1_sb[:], w1x[ht])
                    ps = ps1.tile([128, CHUNK], F32)
                    for dt in range(DT):
                        nc.tensor.matmul(
                            ps[:], w1_sb[:, dt, :], buf_sb[:, dt, :],
                            start=(dt == 0), stop=(dt == DT - 1),
                        )
                    nc.scalar.activation(
                        hT[:, ht, :], ps[:], AF.Relu, bias=b1_sb[:, ht:ht + 1])

                # ---- layer 2: out[dt] = sum_ht w2[ht,dt]^T @ hT[ht] + b2 ----
                for dq in range(DQ):
                    pss = [
                        ps2.tile([128, CHUNK], F32, name=f"pso_{dq}_{i}", tag="pso")
                        for i in range(4)
                    ]
                    for ht in range(HT):
                        w2_sb = w2p.tile([128, 4, 128], BF16)
                        nc.sync.dma_start(w2_sb[:], w2x[ht, :, dq])
                        for i in range(4):
                            nc.tensor.matmul(
                                pss[i][:], w2_sb[:, i, :], hT[:, ht, :],
                                start=(ht == 0), stop=(ht == HT - 1),
                            )
                    for i in range(4):
                        dt = dq * 4 + i
                        o_sb = outp.tile([128, CHUNK], F32)
                        nc.scalar.activation(
                            o_sb[:], pss[i][:], AF.Identity,
                            bias=b2_sb[:, dt:dt + 1])
                        nc.sync.dma_start(outx[cc, dt], o_sb[:])
    nc.compile()
    return nc


def _get_nc():
    if "nc" not in _CACHE:
        _CACHE["nc"] = _build_nc()
    return _CACHE["nc"]


def _route(x_flat, gating_w):
    """Gating softmax + top-k, replicating the reference's jax ops (same
    backend) so routing decisions match bitwise. Falls back to float64
    numpy if jax is unavailable."""
    try:
        import jax
        import jax.numpy as jnp

        gates = jax.nn.softmax(jnp.asarray(x_flat) @ jnp.asarray(gating_w), axis=-1)
        topk_w, topk_idx = jax.lax.top_k(gates, TOP_K)
        norm_w = topk_w / (jnp.sum(topk_w, axis=-1, keepdims=True) + 1e-8)
        return (np.asarray(topk_idx, dtype=np.int64),
                np.asarray(norm_w, dtype=np.float32))
    except Exception:
        logits = x_flat.astype(np.float64) @ gating_w.astype(np.float64)
        m = logits.max(axis=-1, keepdims=True)
        e = np.exp(logits - m)
        gates = (e / e.sum(axis=-1, keepdims=True)).astype(np.float32)
        # top-k with ties broken toward lower index, descending order
        order = np.argsort(-gates, axis=-1, kind="stable")
        topk_idx = order[:, :TOP_K]
        topk_w = np.take_along_axis(gates, topk_idx, axis=-1)
        norm_w = topk_w / (topk_w.sum(axis=-1, keepdims=True) + 1e-8)
        return topk_idx.astype(np.int64), norm_w.astype(np.float32)


def kernel(x, gating_w, w1, b1, w2, b2, **run_kwargs):
    x = np.ascontiguousarray(np.asarray(x, dtype=np.float32))
    gating_w = np.asarray(gating_w, dtype=np.float32)
    w1 = np.asarray(w1, dtype=np.float32)
    b1 = np.asarray(b1, dtype=np.float32)
    w2 = np.asarray(w2, dtype=np.float32)
    b2 = np.asarray(b2, dtype=np.float32)

    x_flat = x.reshape(T, D)

    # ---- routing (host) ----
    topk_idx, norm_w = _route(x_flat, gating_w)
    flat_e = topk_idx.reshape(-1)                       # [T*K]
    flat_t = np.repeat(np.arange(T, dtype=np.int64), TOP_K)
    flat_w = norm_w.reshape(-1)

    onehot = (flat_e[:, None] == np.arange(NUM_EXPERTS)[None, :]).astype(np.int32)
    pos_all = np.cumsum(onehot, axis=0) - 1
    position = pos_all[np.arange(T * TOP_K), flat_e]
    valid = position < CAP

    # ---- dispatch (host side of the "all-to-all") ----
    buf = np.zeros((NUM_EXPERTS, CAP, D), dtype=np.float32)
    buf[flat_e[valid], position[valid]] = x_flat[flat_t[valid]]

    # ---- per-core input packing ----
    in_maps = []
    for e in range(NUM_EXPERTS):
        bufx = (buf[e].reshape(CC, CHUNK, DT, 128).transpose(0, 3, 2, 1)
                .astype(ml_dtypes.bfloat16))
        w1x = (w1[e].reshape(DT, 128, HT, 128).transpose(2, 1, 0, 3)
               .astype(ml_dtypes.bfloat16))
        w2x = (w2[e].reshape(HT, 128, DQ, 4, 128)
               .astype(ml_dtypes.bfloat16))
        b1x = np.ascontiguousarray(b1[e].reshape(HT, 128).T)
        b2x = np.ascontiguousarray(b2[e].reshape(DT, 128).T)
        in_maps.append({
            "bufx": np.ascontiguousarray(bufx),
            "w1x": np.ascontiguousarray(w1x),
            "w2x": np.ascontiguousarray(w2x),
            "b1x": b1x, "b2x": b2x,
        })

    # ---- run expert FFNs on the 8 cores ----
    nc = _get_nc()
    res = bass_utils.run_bass_kernel_spmd(
        nc, in_maps, core_ids=list(range(NUM_EXPERTS)), **run_kwargs)
    if run_kwargs.get("trace"):
        _CACHE["last_results"] = res

    out_all = np.empty((NUM_EXPERTS, CAP, D), dtype=np.float32)
    for e in range(NUM_EXPERTS):
        out_all[e] = (res.results[e]["outx"].transpose(0, 3, 1, 2)
                      .reshape(CAP, D))

    # ---- combine (host side of the "all-to-all" + weighted scatter-add) ----
    pos_g = np.minimum(position, CAP - 1)
    gathered = out_all[flat_e, pos_g]                   # [T*K, D]
    w_eff = np.where(valid, flat_w, 0.0).astype(np.float32)
    out_flat = (gathered * w_eff[:, None]).reshape(T, TOP_K, D).sum(axis=1)
    return out_flat.reshape(B, S, D).astype(np.float32)
